# revision 18
# baseline (speedup 1.0000x reference)
"""AxialSelfAttention2d Trainium2 kernel (8 NeuronCores).

Sharding: stage 1 (row attention, attends along L) is S-sharded (32 rows/core);
stage 2 (column attention, attends along S) is L-sharded (32 cols/core).
Between stages an AllToAll reshards out1 = x + row_out (bf16 payload).

Per-core stage structure ("rows" = s for stage 1, l for stage 2; the attended
axis is 256 long):
  - QKV 1x1-conv projection as matmuls; q/k in [chan, pix] layout (fp32r),
    v projected transposed ([pix, chan]), evacuated PSUM->SBUF on gpsimd.
  - Per (head): QK logits for both rows (fp32r, no max subtraction -- logits
    bounded ~|10|), one fused exp per row on ACT, AV accumulating into a
    [64, 512] PSUM tile (row r in columns r*256:...), ones-matmul softmax
    denominators broadcast over 64 partitions, one reciprocal + one fused
    strided DVE mul per head writing normalized outputs into staging.
  - Residual folded in on gpsimd per chunk.
DMA: all HBM-touching transfers have >=512B contiguous runs. Stage-1 staging
is [c, l, s]-ordered so a2a_in[j, c, g, l, s] group-stores are contiguous;
stage-2 x lives SBUF-resident in xbuf [c, l, s] (bf16), loaded from a2a_out
in 32 bulk contiguous DMAs; y is [c, l_local, s] (host transposes).
"""

import numpy as np
import concourse.bass as bass
import concourse.tile as tile
import concourse.mybir as mybir
from concourse import bacc
from concourse.bass_utils import run_bass_kernel_spmd

N_CORES = 8
D = 512                 # embed channels
H = 8                   # heads
DH = 64                 # head dim
S = 256
L = 256
RLOC = 32               # rows per core (s-rows stage 1, l-cols stage 2)
PIX = RLOC * 256        # 8192 pixels per core per stage
F32 = mybir.dt.float32
F32R = mybir.dt.float32r
BF16 = mybir.dt.bfloat16
ADD = mybir.AluOpType.add

_CACHE = {}


def _load_weights(nc, sb, prefix, w_ins, tag=None, bf16=False):
    """DMA weight/bias DRAM inputs into SBUF tiles. Returns dict of tiles.

    With bf16=True the w/bvr DRAM tensors are bf16 and land in the front
    half of the same (tag-aliased) f32r-sized buffers via bitcast views.
    """
    wq_d, wk_d, wv_d, bq_d, bk_d, bv_d = w_ins
    out = {}
    for wname, wd in (("wq", wq_d), ("wk", wk_d), ("wv", wv_d)):
        tiles = []
        for c4 in range(4):
            t = sb.tile([128, 512], F32R, name=f"{prefix}{wname}{c4}",
                        tag=(f"{tag}{wname}{c4}" if tag else None),
                        bufs=1)
            v = t[:].bitcast(BF16)[:, 0:512] if bf16 else t[:]
            nc.sync.dma_start(v, wd[c4 * 128:(c4 + 1) * 128, :])
            tiles.append(v)
        out[wname] = tiles
    for bname, bd in (("bq", bq_d), ("bk", bk_d)):
        tiles = []
        for oc in range(4):
            t = sb.tile([128, 1], F32, name=f"{prefix}{bname}{oc}",
                        tag=(f"{tag}{bname}{oc}" if tag else None), bufs=1)
            nc.sync.dma_start(t[:], bd[oc * 128:(oc + 1) * 128, :])
            tiles.append(t[:])
        out[bname] = tiles
    bvr = sb.tile([1, 512], F32R, name=f"{prefix}bvr",
                  tag=(f"{tag}bvr" if tag else None), bufs=1)
    if bf16:
        bv = bvr[:].bitcast(BF16)[:, 0:512]
        nc.sync.dma_start(bv, bv_d.rearrange("h d one -> one (h d)"))
        out["bvr"] = bv
    else:
        nc.sync.dma_start(bvr[:],
                          bv_d.rearrange("h d one -> one (h d)").bitcast(F32R))
        out["bvr"] = bvr[:]
    return out


def _stage(tc, nc, sb, ps, w, consts, x_get, stag_new, out_store,
           prefix):
    """One attention stage over this core's 32 rows.

    x_get(cc, chunk) -> ([128, 512] x-tile AP, is_bf16): chunk's pixels.
    stag_new(cc) -> staging tile for one 8-row group.
    out_store(cc, g, stag_tile): emit finished group staging.
    Staging layout: stage 1 [128c, 256l, 8s]; stage 2 [128c, 8l, 256s].
    The fused normalize mul writes per-head [64, 256row, 2r] views; dest
    strides differ per stage and are derived from the tile shape.
    """
    onescol = consts
    for g in range(4):                      # groups of 8 rows
        stag = [stag_new(cc) for cc in range(4)]
        for c2 in range(4):                 # 2-row chunks within group
            chunk = g * 4 + c2
            x_t = [x_get(cc, chunk) for cc in range(4)]
            # --- q/k projections: out [o-chunk 128, 512 pix] ---
            q_sb, k_sb = [], []
            for wname, bname, dst in (("wq", "bq", q_sb), ("wk", "bk", k_sb)):
                for oc in range(4):
                    pp = ps["ps"].tile([128, 512], F32, name="pp", tag="pp",
                                       bufs=2)
                    for c4 in range(4):
                        nc.tensor.matmul(
                            pp[:],
                            w[wname][c4][:, oc * 128:(oc + 1) * 128],
                            x_t[c4],
                            start=(c4 == 0), stop=(c4 == 3),
                        )
                    t = sb.tile([128, 512], F32R, name=f"{wname}o{oc}",
                                tag=f"{wname}o", bufs=4)
                    nc.scalar.activation(
                        t[:], pp[:], mybir.ActivationFunctionType.Identity,
                        bias=w[bname][oc][:],
                    )
                    dst.append(t)
            # --- v projected transposed [pix-chunk 128, 8 heads x 64],
            #     v-bias added via a K=1 ones x bvr matmul; PSUM->SBUF
            #     evacuation on gpsimd ---
            vT_sb = []
            for pc in range(4):
                pv = ps["ps"].tile([128, 512], F32, name="pp", tag="pp",
                                   bufs=2)
                for c4 in range(4):
                    nc.tensor.matmul(
                        pv[:],
                        x_t[c4][:, pc * 128:(pc + 1) * 128],
                        w["wv"][c4][:],
                        start=(c4 == 0), stop=False,
                    )
                nc.tensor.matmul(
                    pv[:], w["ones"], w["bvr"],
                    start=False, stop=True,
                )
                t = sb.tile([128, 512], F32R, name=f"vT{pc}", tag="vT",
                            bufs=4)
                with nc.allow_low_precision(reason="fp32r staging"):
                    nc.vector.tensor_copy(t[:], pv[:])
                vT_sb.append(t)

            # --- attention per head, r-paired, software-pipelined:
            #     QK+exp of head h+1 emitted before normalize of head h ---
            def emit_qkexp(h):
                m, ph = h // 2, (h % 2) * 64
                e_t = []
                for r in range(2):
                    at = ps["ps"].tile([128, 512], F32, name="at", tag="at",
                                       bufs=2)
                    for jh in range(2):
                        nc.tensor.matmul(
                            at[:, jh * 256:(jh + 1) * 256],
                            k_sb[m][ph:ph + 64,
                                    r * 256 + jh * 128:
                                    r * 256 + (jh + 1) * 128],
                            q_sb[m][ph:ph + 64, r * 256:(r + 1) * 256],
                            start=True, stop=True,
                        )
                    e = sb.tile([128, 512], F32R, name="e_t", tag="e_t",
                                bufs=3)
                    nc.scalar.activation(
                        e[:], at[:], mybir.ActivationFunctionType.Exp)
                    e_t.append(e)
                return e_t

            def emit_av(h, e_t):
                # ob rows = attn @ v (unnormalized), columns r*256:... per r;
                # dn = softmax denominators broadcast over 64 partitions
                ob = ps["ps"].tile([64, 512], F32, name="ob", tag="ob",
                                   bufs=2)
                dn = ps["ps"].tile([64, 512], F32, name="dn", tag="dn",
                                   bufs=2)
                for r in range(2):
                    for jh in range(2):
                        nc.tensor.matmul(
                            ob[:, r * 256:(r + 1) * 256],
                            vT_sb[2 * r + jh][:, h * 64:h * 64 + 64],
                            e_t[r][:, jh * 256:(jh + 1) * 256],
                            start=(jh == 0), stop=(jh == 1),
                        )
                for r in range(2):
                    for jh in range(2):
                        nc.tensor.matmul(
                            dn[:, r * 256:(r + 1) * 256],
                            onescol[:, 0:64],
                            e_t[r][:, jh * 256:(jh + 1) * 256],
                            start=(jh == 0), stop=(jh == 1),
                        )
                r_sb = sb.tile([64, 512], F32, name="r_sb", tag="r_sb",
                               bufs=2)
                nc.vector.reciprocal(r_sb[:], dn[:])
                return ob, r_sb

            def emit_norm(h, ob, r_sb):
                m, ph = h // 2, (h % 2) * 64
                # strided (row, r) views: ob/r_sb columns are r*256 + i
                st = stag[m]
                if st.shape[1] == 256:      # stage 1: [c, 256l, 8s]
                    dst = st[ph:ph + 64, :, c2 * 2:c2 * 2 + 2]
                else:                       # stage 2: [c, 8l, 256s]
                    dst = st[ph:ph + 64, c2 * 2:c2 * 2 + 2, :] \
                        .rearrange("p r i -> p i r")
                src = ob[:].rearrange("p (r i) -> p i r", r=2)
                rcv = r_sb[:].rearrange("p (r i) -> p i r", r=2)
                with nc.allow_low_precision(reason="staging dtype"):
                    nc.vector.tensor_mul(dst, src, rcv)

            pending = None
            for h in range(H):
                e_t = emit_qkexp(h)
                ob, r_sb = emit_av(h, e_t)
                if pending is not None:
                    emit_norm(*pending)
                pending = (h, ob, r_sb)
            emit_norm(*pending)
            # bulk residual: stag[cc] group slice += x (on gpsimd)
            for cc in range(4):
                st = stag[cc]
                if st.shape[1] == 256:      # stage 1
                    dst = st[:, :, c2 * 2:c2 * 2 + 2]
                    xv = x_t[cc].rearrange("c (r i) -> c i r", r=2)
                else:                       # stage 2
                    dst = st[:, c2 * 2:c2 * 2 + 2, :]
                    xv = x_t[cc].rearrange("c (r i) -> c r i", r=2)
                with nc.allow_low_precision(reason="staging dtype"):
                    nc.gpsimd.tensor_add(dst, dst, xv)
        for cc in range(4):
            out_store(cc, g, stag[cc])


def _build(variant="full"):
    ndev = 1 if variant == "sim1" else N_CORES
    nc = bacc.Bacc("TRN2", target_bir_lowering=False, debug=False,
                   num_devices=ndev)
    if variant == "noop":
        xi = nc.dram_tensor("xi", [128, 512], F32, kind="ExternalInput").ap()
        y = nc.dram_tensor("y", [128, 512], F32, kind="ExternalOutput").ap()
        with tile.TileContext(nc) as tc:
            with tc.tile_pool(name="sb", bufs=1) as sb:
                t = sb.tile([128, 512], F32, name="t")
                nc.sync.dma_start(t[:], xi[:])
                nc.sync.dma_start(y[:], t[:])
        nc.compile()
        return nc

    xi = nc.dram_tensor("xi", [D, PIX], F32R, kind="ExternalInput").ap()
    y = nc.dram_tensor("y", [D, RLOC, S], F32, kind="ExternalOutput").ap()
    w_ins = {}
    for p in ("1", "2"):
        ins = []
        for nm, shp in (("wq", [D, D]), ("wk", [D, D]), ("wv", [D, D]),
                        ("bq", [D, 1]), ("bk", [D, 1]), ("bv", [H, DH, 1])):
            if nm in ("bq", "bk"):
                dt = F32
            elif p == "2" and nm in ("wq", "wk", "wv", "bv"):
                dt = BF16
            else:
                dt = F32R
            ins.append(nc.dram_tensor(nm + p, shp, dt, kind="ExternalInput").ap())
        w_ins[p] = ins

    n_rep = {"full3": 3, "full8": 8, "noa2a8": 8}.get(variant, 1)
    use_a2a = variant not in ("noa2a", "noa2a8", "sim1")

    with tile.TileContext(nc) as tc:
        with tc.tile_pool(name="sb", bufs=1) as sb, \
             tc.tile_pool(name="psum", bufs=1, space="PSUM") as psp, \
             tc.tile_pool(name="dram", bufs=1, space="DRAM") as dram:
            ps = {"ps": psp, "sb": sb}
            # a2a blocks: [dest core j][c][group][l local to j][s in group]
            a2a_in = dram.tile([N_CORES, D, 4, RLOC, 8], BF16, name="a2a_in")
            a2a_out = dram.tile([N_CORES, D, 4, RLOC, 8], BF16, name="a2a_out")

            ones_sb = sb.tile([1, 128], F32R, name="ones_sb", bufs=1)
            nc.gpsimd.memset(ones_sb[:].bitcast(mybir.dt.uint32), 0x3F800000)
            ones_bf = sb.tile([1, 128], BF16, name="ones_bf", bufs=1)
            nc.gpsimd.memset(ones_bf[:].bitcast(mybir.dt.uint16), 0x3F80)
            onescol = sb.tile([128, 64], F32R, name="onescol", bufs=1)
            nc.gpsimd.memset(onescol[:].bitcast(mybir.dt.uint32), 0x3F800000)
            consts = onescol

            w1 = _load_weights(nc, sb, "s1", w_ins["1"], tag="w")
            w1["ones"] = ones_sb[0:1, :]
            # xbuf: stage-2 x resident [c, l local, s] in bf16
            xbuf = [sb.tile([128, RLOC, S], BF16, name=f"xbuf{cc}", bufs=1)
                    for cc in range(4)]

            # ---- stage 1: row attention, S-sharded ----
            def x_get1(cc, chunk):
                t = sb.tile([128, 512], F32R, name=f"x{cc}", tag=f"x{cc}",
                            bufs=2)
                nc.sync.dma_start(
                    t[:], xi[cc * 128:(cc + 1) * 128,
                             chunk * 512:(chunk + 1) * 512])
                return t[:]

            def stag_new1(cc):
                return ps["sb"].tile([128, 256, 8], BF16, name=f"s1g{cc}",
                                     tag=f"s1g{cc}", bufs=2)

            def out_store1(cc, g, stg):
                # [c, 256l, 8s] -> a2a_in[j, c, g, l32, s8]; contiguous
                # 512B runs per partition on the HBM side
                dst = a2a_in[:, cc * 128:(cc + 1) * 128, g, :, :] \
                    .transpose([1, 0, 2, 3])
                nc.scalar.dma_start(
                    dst, stg[:].rearrange("c (j l) s -> c j l s", j=N_CORES))

            def reshard():
                if not use_a2a:
                    for j in range(N_CORES):
                        nc.gpsimd.dma_start(a2a_out[j], a2a_in[j])
                else:
                    nc.gpsimd.collective_compute(
                        "AllToAll", mybir.AluOpType.bypass,
                        replica_groups=[list(range(N_CORES))],
                        ins=[a2a_in.opt()], outs=[a2a_out.opt()],
                    )

            def load_xbuf():
                # a2a_out[i, c, g, l, s8] -> xbuf[c, l, i*32 + g*8 + s]
                for i in range(N_CORES):
                    for cc in range(4):
                        for gg in range(4):
                            o = i * 32 + gg * 8
                            nc.sync.dma_start(
                                xbuf[cc][:, :, o:o + 8],
                                a2a_out[i, cc * 128:(cc + 1) * 128, gg, :, :])

            # ---- stage 2: column attention, L-sharded ----
            def x_get2(cc, chunk):
                return xbuf[cc][:, chunk * 2:chunk * 2 + 2, :] \
                    .rearrange("c r i -> c (r i)")

            def stag_new2(cc):
                return ps["sb"].tile([128, 8, 256], F32, name=f"s2g{cc}",
                                     tag=f"s2g{cc}", bufs=1)

            def out_store2(cc, g, stg):
                nc.scalar.dma_start(
                    y[cc * 128:(cc + 1) * 128, g * 8:(g + 1) * 8, :], stg[:])

            for rep in range(n_rep):
                _stage(tc, nc, sb, ps, w1, consts, x_get1,
                       stag_new1, out_store1, "s1")
                reshard()
                if rep == 0:
                    w2 = _load_weights(nc, sb, "s2", w_ins["2"], tag="w",
                                       bf16=True)
                    w2["ones"] = ones_bf[0:1, :]
                load_xbuf()
                _stage(tc, nc, sb, ps, w2, consts, x_get2,
                       stag_new2, out_store2, "s2")

    nc.compile()
    return nc


def _get_nc(variant="full"):
    key = "nc:" + variant
    if key not in _CACHE:
        _CACHE[key] = _build(variant)
    return _CACHE[key]


def _in_maps(x, Wr, br, Wc, bc):
    import ml_dtypes
    x = np.asarray(x, dtype=np.float32)
    stage_w = {}
    for p, W, b in (("1", np.asarray(Wr, np.float32), np.asarray(br, np.float32)),
                    ("2", np.asarray(Wc, np.float32), np.asarray(bc, np.float32))):
        wdt = ml_dtypes.bfloat16 if p == "2" else np.float32
        stage_w["wq" + p] = np.ascontiguousarray(W[0:D].T.astype(wdt))
        stage_w["wk" + p] = np.ascontiguousarray(W[D:2 * D].T.astype(wdt))
        stage_w["wv" + p] = np.ascontiguousarray(W[2 * D:3 * D].T.astype(wdt))
        stage_w["bq" + p] = np.ascontiguousarray(b[0:D].reshape(D, 1))
        stage_w["bk" + p] = np.ascontiguousarray(b[D:2 * D].reshape(D, 1))
        stage_w["bv" + p] = np.ascontiguousarray(
            b[2 * D:3 * D].reshape(H, DH, 1).astype(wdt))
    maps = []
    for i in range(N_CORES):
        m = {"xi": np.ascontiguousarray(
            x[0, :, i * RLOC:(i + 1) * RLOC, :].reshape(D, PIX))}
        m.update(stage_w)
        maps.append(m)
    return maps


def _get_runner(variant="full"):
    """Build (once) a cached jitted shard_map callable over the 8 cores."""
    rkey = "runner:" + variant
    if rkey in _CACHE:
        return _CACHE[rkey]
    import jax
    from jax.sharding import Mesh, PartitionSpec
    from jax.experimental.shard_map import shard_map
    from concourse import bass2jax as b2j

    nc = _get_nc(variant)
    b2j.install_neuronx_cc_hook()
    part_name = nc.partition_id_tensor.name if nc.partition_id_tensor else None
    in_names, out_names, out_avals, zero_outs = [], [], [], []
    for alloc in nc.m.functions[0].allocations:
        if not isinstance(alloc, mybir.MemoryLocationSet):
            continue
        name = alloc.memorylocations[0].name
        if alloc.kind == "ExternalInput":
            if name != part_name:
                in_names.append(name)
        elif alloc.kind == "ExternalOutput":
            out_names.append(name)
            shape = tuple(alloc.tensor_shape)
            dtype = mybir.dt.np(alloc.dtype)
            out_avals.append(jax.core.ShapedArray(shape, dtype))
            zero_outs.append(np.zeros(shape, dtype))
    n_params = len(in_names)
    all_names = in_names + out_names
    if part_name is not None:
        all_names = all_names + [part_name]

    def _body(*args):
        operands = list(args)
        if part_name is not None:
            operands.append(b2j.partition_id_tensor())
        outs = b2j._bass_exec_p.bind(
            *operands,
            out_avals=tuple(out_avals),
            in_names=tuple(all_names),
            out_names=tuple(out_names),
            lowering_input_output_aliases=(),
            sim_require_finite=True,
            sim_require_nnan=True,
            nc=nc,
        )
        return tuple(outs)

    devices = jax.devices()[:N_CORES]
    mesh = Mesh(np.asarray(devices), ("core",))
    specs = (PartitionSpec("core"),) * (n_params + len(out_names))
    sharded = jax.jit(
        shard_map(_body, mesh=mesh, in_specs=specs,
                  out_specs=(PartitionSpec("core"),) * len(out_names),
                  check_rep=False),
        keep_unused=True,
    )
    concat_zeros = [
        jax.device_put(
            np.zeros((N_CORES * z.shape[0], *z.shape[1:]), z.dtype),
            jax.sharding.NamedSharding(mesh, PartitionSpec("core")))
        for z in zero_outs
    ]
    _CACHE[rkey] = (sharded, in_names, out_names, out_avals, concat_zeros)
    return _CACHE[rkey]


def _run(maps):
    sharded, in_names, out_names, out_avals, concat_zeros = _get_runner()
    concat_in = [
        np.concatenate([maps[c][nm] for c in range(N_CORES)], axis=0)
        for nm in in_names
    ]
    out_arrs = sharded(*concat_in, *concat_zeros)
    return [
        {nm: np.asarray(out_arrs[i]).reshape(N_CORES, *out_avals[i].shape)[c]
         for i, nm in enumerate(out_names)}
        for c in range(N_CORES)
    ]


def kernel(x, Wr, br, Wc, bc):
    maps = _in_maps(x, Wr, br, Wc, bc)
    results = _run(maps)
    # y per core is [c, l_local, s] -> [c, s, l_local], concat over cores on l
    out = np.concatenate(
        [results[i]["y"].transpose(0, 2, 1) for i in range(N_CORES)], axis=2)
    return out[None].astype(np.float32)


# revision 30
# speedup vs baseline: 1.0493x; 1.0493x over previous
"""AxialSelfAttention2d Trainium2 kernel (8 NeuronCores).

Sharding: stage 1 (row attention, attends along L) is S-sharded (32 rows/core);
stage 2 (column attention, attends along S) is L-sharded (32 cols/core).
Between stages an AllToAll reshards out1 = x + row_out (bf16 payload).

Per-core stage structure ("rows" = s for stage 1, l for stage 2; the attended
axis is 256 long):
  - QKV 1x1-conv projection as matmuls; q/k in [chan, pix] layout (fp32r),
    v projected transposed ([pix, chan]), evacuated PSUM->SBUF on gpsimd.
  - Per (head): QK logits for both rows (fp32r, no max subtraction -- logits
    bounded ~|10|), one fused exp per row on ACT, AV accumulating into a
    [64, 512] PSUM tile (row r in columns r*256:...), ones-matmul softmax
    denominators broadcast over 64 partitions, one reciprocal + one fused
    strided DVE mul per head writing normalized outputs into staging.
  - Residual folded in on gpsimd per chunk.
DMA: all HBM-touching transfers have >=512B contiguous runs. Stage-1 staging
is [c, l, s]-ordered so a2a_in[j, c, g, l, s] group-stores are contiguous;
stage-2 x lives SBUF-resident in xbuf [c, l, s] (bf16), loaded from a2a_out
in 32 bulk contiguous DMAs; y is [c, l_local, s] (host transposes).
"""

import numpy as np
import concourse.bass as bass
import concourse.tile as tile
import concourse.mybir as mybir
from concourse import bacc
from concourse.bass_utils import run_bass_kernel_spmd

N_CORES = 8
D = 512                 # embed channels
H = 8                   # heads
DH = 64                 # head dim
S = 256
L = 256
RLOC = 32               # rows per core (s-rows stage 1, l-cols stage 2)
PIX = RLOC * 256        # 8192 pixels per core per stage
F32 = mybir.dt.float32
F32R = mybir.dt.float32r
BF16 = mybir.dt.bfloat16
ADD = mybir.AluOpType.add

_CACHE = {}


def _load_weights(nc, sb, prefix, w_ins, tag=None, bf16=False):
    """DMA weight/bias DRAM inputs into SBUF tiles. Returns dict of tiles.

    With bf16=True the w/bvr DRAM tensors are bf16 and land in the front
    half of the same (tag-aliased) f32r-sized buffers via bitcast views.
    """
    wq_d, wk_d, wv_d, bq_d, bk_d, bv_d = w_ins
    out = {}
    for wname, wd in (("wq", wq_d), ("wk", wk_d), ("wv", wv_d)):
        tiles = []
        for c4 in range(4):
            t = sb.tile([128, 512], F32R, name=f"{prefix}{wname}{c4}",
                        tag=(f"{tag}{wname}{c4}" if tag else None),
                        bufs=1)
            v = t[:].bitcast(BF16)[:, 0:512] if bf16 else t[:]
            nc.scalar.dma_start(v, wd[c4 * 128:(c4 + 1) * 128, :])
            tiles.append(v)
        out[wname] = tiles
    for bname, bd in (("bq", bq_d), ("bk", bk_d)):
        tiles = []
        for oc in range(4):
            t = sb.tile([128, 1], F32, name=f"{prefix}{bname}{oc}",
                        tag=(f"{tag}{bname}{oc}" if tag else None), bufs=1)
            nc.scalar.dma_start(t[:], bd[oc * 128:(oc + 1) * 128, :])
            tiles.append(t[:])
        out[bname] = tiles
    bvr = sb.tile([1, 512], F32R, name=f"{prefix}bvr",
                  tag=(f"{tag}bvr" if tag else None), bufs=1)
    if bf16:
        bv = bvr[:].bitcast(BF16)[:, 0:512]
        nc.scalar.dma_start(bv, bv_d.rearrange("h d one -> one (h d)"))
        out["bvr"] = bv
    else:
        nc.scalar.dma_start(bvr[:],
                            bv_d.rearrange("h d one -> one (h d)").bitcast(F32R))
        out["bvr"] = bvr[:]
    return out


def _stage(tc, nc, sb, ps, w, consts, x_get, stag_new, out_store,
           prefix):
    """One attention stage over this core's 32 rows.

    x_get(cc, chunk) -> ([128, 512] x-tile AP, is_bf16): chunk's pixels.
    stag_new(cc) -> staging tile for one 8-row group.
    out_store(cc, g, stag_tile): emit finished group staging.
    Staging layout: stage 1 [128c, 256l, 8s]; stage 2 [128c, 8l, 256s].
    The fused normalize mul writes per-head [64, 256row, 2r] views; dest
    strides differ per stage and are derived from the tile shape.
    """
    onescol, group_done = consts
    for g in range(4):                      # groups of 8 rows
        stag = [stag_new(cc) for cc in range(4)]
        for c2 in range(4):                 # 2-row chunks within group
            chunk = g * 4 + c2
            x_t = [x_get(cc, chunk) for cc in range(4)]
            # --- q/k projections: out [o-chunk 128, 512 pix] ---
            q_sb, k_sb = [], []
            for wname, bname, dst in (("wq", "bq", q_sb), ("wk", "bk", k_sb)):
                for oc in range(4):
                    pp = ps["ps"].tile([128, 512], F32, name="pp", tag="pp",
                                       bufs=2)
                    for c4 in range(4):
                        nc.tensor.matmul(
                            pp[:],
                            w[wname][c4][:, oc * 128:(oc + 1) * 128],
                            x_t[c4],
                            start=(c4 == 0), stop=(c4 == 3),
                        )
                    t = sb.tile([128, 512], F32R, name=f"{wname}o{oc}",
                                tag=f"{wname}o", bufs=4)
                    nc.scalar.activation(
                        t[:], pp[:], mybir.ActivationFunctionType.Identity,
                        bias=w[bname][oc][:],
                    )
                    dst.append(t)
            # --- v projected transposed [pix-chunk 128, 8 heads x 64],
            #     v-bias added via a K=1 ones x bvr matmul; PSUM->SBUF
            #     evacuation on gpsimd ---
            vT_sb = []
            for pc in range(4):
                pv = ps["ps"].tile([128, 512], F32, name="pp", tag="pp",
                                   bufs=2)
                for c4 in range(4):
                    nc.tensor.matmul(
                        pv[:],
                        x_t[c4][:, pc * 128:(pc + 1) * 128],
                        w["wv"][c4][:],
                        start=(c4 == 0), stop=False,
                    )
                nc.tensor.matmul(
                    pv[:], w["ones"], w["bvr"],
                    start=False, stop=True,
                )
                t = sb.tile([128, 512], F32R, name=f"vT{pc}", tag="vT",
                            bufs=4)
                with nc.allow_low_precision(reason="fp32r staging"):
                    nc.vector.tensor_copy(t[:], pv[:])
                vT_sb.append(t)

            # --- attention per head, r-paired, software-pipelined:
            #     QK+exp of head h+1 emitted before normalize of head h ---
            def emit_qkexp(h):
                m, ph = h // 2, (h % 2) * 64
                e_t = []
                for r in range(2):
                    at = ps["ps"].tile([128, 512], F32, name="at", tag="at",
                                       bufs=2)
                    for jh in range(2):
                        nc.tensor.matmul(
                            at[:, jh * 256:(jh + 1) * 256],
                            k_sb[m][ph:ph + 64,
                                    r * 256 + jh * 128:
                                    r * 256 + (jh + 1) * 128],
                            q_sb[m][ph:ph + 64, r * 256:(r + 1) * 256],
                            start=True, stop=True,
                        )
                    e = sb.tile([128, 512], F32R, name="e_t", tag="e_t",
                                bufs=3)
                    nc.scalar.activation(
                        e[:], at[:], mybir.ActivationFunctionType.Exp)
                    e_t.append(e)
                return e_t

            def emit_av(h, e_t):
                # ob rows = attn @ v (unnormalized), columns r*256:... per r;
                # dn = softmax denominators broadcast over 64 partitions
                ob = ps["ps"].tile([64, 512], F32, name="ob", tag="ob",
                                   bufs=2)
                dn = ps["ps"].tile([64, 512], F32, name="dn", tag="dn",
                                   bufs=2)
                for r in range(2):
                    for jh in range(2):
                        nc.tensor.matmul(
                            ob[:, r * 256:(r + 1) * 256],
                            vT_sb[2 * r + jh][:, h * 64:h * 64 + 64],
                            e_t[r][:, jh * 256:(jh + 1) * 256],
                            start=(jh == 0), stop=(jh == 1),
                        )
                for r in range(2):
                    for jh in range(2):
                        nc.tensor.matmul(
                            dn[:, r * 256:(r + 1) * 256],
                            onescol[:, 0:64],
                            e_t[r][:, jh * 256:(jh + 1) * 256],
                            start=(jh == 0), stop=(jh == 1),
                        )
                r_sb = sb.tile([64, 512], F32, name="r_sb", tag="r_sb",
                               bufs=2)
                nc.vector.reciprocal(r_sb[:], dn[:])
                return ob, r_sb

            def emit_norm(h, ob, r_sb):
                m, ph = h // 2, (h % 2) * 64
                # strided (row, r) views: ob/r_sb columns are r*256 + i
                st = stag[m]
                if st.shape[1] == 256:      # stage 1: [c, 256l, 8s]
                    dst = st[ph:ph + 64, :, c2 * 2:c2 * 2 + 2]
                else:                       # stage 2: [c, 8l, 256s]
                    dst = st[ph:ph + 64, c2 * 2:c2 * 2 + 2, :] \
                        .rearrange("p r i -> p i r")
                src = ob[:].rearrange("p (r i) -> p i r", r=2)
                rcv = r_sb[:].rearrange("p (r i) -> p i r", r=2)
                with nc.allow_low_precision(reason="staging dtype"):
                    nc.vector.tensor_mul(dst, src, rcv)

            pending = None
            for h in range(H):
                e_t = emit_qkexp(h)
                ob, r_sb = emit_av(h, e_t)
                if pending is not None:
                    emit_norm(*pending)
                pending = (h, ob, r_sb)
            emit_norm(*pending)
            # bulk residual: stag[cc] group slice += x (on gpsimd)
            for cc in range(4):
                st = stag[cc]
                if st.shape[1] == 256:      # stage 1
                    dst = st[:, :, c2 * 2:c2 * 2 + 2]
                    xv = x_t[cc].rearrange("c (r i) -> c i r", r=2)
                else:                       # stage 2
                    dst = st[:, c2 * 2:c2 * 2 + 2, :]
                    xv = x_t[cc].rearrange("c (r i) -> c r i", r=2)
                with nc.allow_low_precision(reason="staging dtype"):
                    nc.gpsimd.tensor_add(dst, dst, xv)
        for cc in range(4):
            out_store(cc, g, stag[cc])
        if group_done is not None:
            group_done(g)


def _build(variant="full"):
    ndev = 1 if variant == "sim1" else N_CORES
    nc = bacc.Bacc("TRN2", target_bir_lowering=False, debug=False,
                   num_devices=ndev)
    if variant == "noop":
        xi = nc.dram_tensor("xi", [128, 512], F32, kind="ExternalInput").ap()
        y = nc.dram_tensor("y", [128, 512], F32, kind="ExternalOutput").ap()
        with tile.TileContext(nc) as tc:
            with tc.tile_pool(name="sb", bufs=1) as sb:
                t = sb.tile([128, 512], F32, name="t")
                nc.sync.dma_start(t[:], xi[:])
                nc.sync.dma_start(y[:], t[:])
        nc.compile()
        return nc

    xi = nc.dram_tensor("xi", [D, PIX], F32R, kind="ExternalInput").ap()
    y = nc.dram_tensor("y", [D, RLOC, S], F32, kind="ExternalOutput").ap()
    w_ins = {}
    for p in ("1", "2"):
        ins = []
        for nm, shp in (("wq", [D, D]), ("wk", [D, D]), ("wv", [D, D]),
                        ("bq", [D, 1]), ("bk", [D, 1]), ("bv", [H, DH, 1])):
            if nm in ("bq", "bk"):
                dt = F32
            elif p == "2" and nm in ("wq", "wk", "wv", "bv"):
                dt = BF16
            else:
                dt = F32R
            ins.append(nc.dram_tensor(nm + p, shp, dt, kind="ExternalInput").ap())
        w_ins[p] = ins

    n_rep = {"full3": 3, "full8": 8, "noa2a8": 8}.get(variant, 1)
    use_a2a = variant not in ("noa2a", "noa2a8", "sim1")

    with tile.TileContext(nc) as tc:
        with tc.tile_pool(name="sb", bufs=1) as sb, \
             tc.tile_pool(name="psum", bufs=1, space="PSUM") as psp, \
             tc.tile_pool(name="dram", bufs=1, space="DRAM") as dram:
            ps = {"ps": psp, "sb": sb}
            # per-group a2a blocks: [dest core j][c][l local to j][s in group]
            a2a_in = [dram.tile([N_CORES, D, RLOC, 8], BF16, name=f"a2a_in{g}")
                      for g in range(4)]
            a2a_out = [dram.tile([N_CORES, D, RLOC, 8], BF16,
                                 name=f"a2a_out{g}") for g in range(4)]

            ones_sb = sb.tile([1, 128], F32R, name="ones_sb", bufs=1)
            nc.gpsimd.memset(ones_sb[:].bitcast(mybir.dt.uint32), 0x3F800000)
            ones_bf = sb.tile([1, 128], BF16, name="ones_bf", bufs=1)
            nc.gpsimd.memset(ones_bf[:].bitcast(mybir.dt.uint16), 0x3F80)
            onescol = sb.tile([128, 64], F32R, name="onescol", bufs=1)
            nc.gpsimd.memset(onescol[:].bitcast(mybir.dt.uint32), 0x3F800000)

            w1 = _load_weights(nc, sb, "s1", w_ins["1"], tag="w")
            w1["ones"] = ones_sb[0:1, :]
            # xbuf: stage-2 x resident [c, l local, s] in bf16
            xbuf = [sb.tile([128, RLOC, S], BF16, name=f"xbuf{cc}", bufs=1)
                    for cc in range(4)]

            # ---- stage 1: row attention, S-sharded ----
            def x_get1(cc, chunk):
                t = sb.tile([128, 512], F32R, name=f"x{cc}", tag=f"x{cc}",
                            bufs=2)
                nc.sync.dma_start(
                    t[:], xi[cc * 128:(cc + 1) * 128,
                             chunk * 512:(chunk + 1) * 512])
                return t[:]

            def stag_new1(cc):
                return ps["sb"].tile([128, 256, 8], BF16, name=f"s1g{cc}",
                                     tag=f"s1g{cc}", bufs=2)

            def out_store1(cc, g, stg):
                # [c, 256l, 8s] -> a2a_in[g][j, c, l32, s8]; contiguous
                # 512B runs per partition on the HBM side
                dst = a2a_in[g][:, cc * 128:(cc + 1) * 128, :, :] \
                    .transpose([1, 0, 2, 3])
                nc.scalar.dma_start(
                    dst, stg[:].rearrange("c (j l) s -> c j l s", j=N_CORES))

            def group_done1(g):
                # reshard group g; overlaps stage-1 compute of later groups
                if not use_a2a:
                    for j in range(N_CORES):
                        nc.gpsimd.dma_start(a2a_out[g][j], a2a_in[g][j])
                else:
                    nc.gpsimd.collective_compute(
                        "AllToAll", mybir.AluOpType.bypass,
                        replica_groups=[list(range(N_CORES))],
                        ins=[a2a_in[g].opt()], outs=[a2a_out[g].opt()],
                    )

            def load_xbuf():
                # a2a_out[g][i, c, l, s8] -> xbuf[c, l, i*32 + g*8 + s],
                # spread across three DMA queues. tile_wait_until keeps the
                # scheduler from hoisting these into the stage-1 queue
                # streams, where their wait-for-collective would block the
                # FIFO sequencers.
                engs = (nc.sync, nc.scalar, nc.gpsimd, nc.vector)
                n = 0
                with tc.tile_wait_until(0.42):
                    for g in range(4):
                        for i in range(N_CORES):
                            for cc in range(4):
                                o = i * 32 + g * 8
                                engs[n % 3].dma_start(
                                    xbuf[cc][:, :, o:o + 8],
                                    a2a_out[g][i, cc * 128:(cc + 1) * 128,
                                               :, :])
                                n += 1

            # ---- stage 2: column attention, L-sharded ----
            def x_get2(cc, chunk):
                return xbuf[cc][:, chunk * 2:chunk * 2 + 2, :] \
                    .rearrange("c r i -> c (r i)")

            def stag_new2(cc):
                return ps["sb"].tile([128, 8, 256], F32, name=f"s2g{cc}",
                                     tag=f"s2g{cc}", bufs=1)

            def out_store2(cc, g, stg):
                nc.scalar.dma_start(
                    y[cc * 128:(cc + 1) * 128, g * 8:(g + 1) * 8, :], stg[:])

            for rep in range(n_rep):
                _stage(tc, nc, sb, ps, w1, (onescol, group_done1), x_get1,
                       stag_new1, out_store1, "s1")
                load_xbuf()
                if rep == 0:
                    if n_rep == 1:
                        w2 = _load_weights(nc, sb, "s2", w_ins["2"], tag="w",
                                           bf16=True)
                    else:
                        # timing probes: stage-2 runs on bitcast views of the
                        # stage-1 weight buffers (garbage values, same timing)
                        w2 = {k: [a.bitcast(BF16)[:, 0:512] for a in v]
                              for k, v in w1.items()
                              if k in ("wq", "wk", "wv")}
                        w2["bq"], w2["bk"] = w1["bq"], w1["bk"]
                        w2["bvr"] = w1["bvr"].bitcast(BF16)[:, 0:512]
                    w2["ones"] = ones_bf[0:1, :]
                _stage(tc, nc, sb, ps, w2, (onescol, None), x_get2,
                       stag_new2, out_store2, "s2")

    nc.compile()
    return nc


def _get_nc(variant="full"):
    key = "nc:" + variant
    if key not in _CACHE:
        _CACHE[key] = _build(variant)
    return _CACHE[key]


def _in_maps(x, Wr, br, Wc, bc):
    import ml_dtypes
    x = np.asarray(x, dtype=np.float32)
    stage_w = {}
    for p, W, b in (("1", np.asarray(Wr, np.float32), np.asarray(br, np.float32)),
                    ("2", np.asarray(Wc, np.float32), np.asarray(bc, np.float32))):
        wdt = ml_dtypes.bfloat16 if p == "2" else np.float32
        stage_w["wq" + p] = np.ascontiguousarray(W[0:D].T.astype(wdt))
        stage_w["wk" + p] = np.ascontiguousarray(W[D:2 * D].T.astype(wdt))
        stage_w["wv" + p] = np.ascontiguousarray(W[2 * D:3 * D].T.astype(wdt))
        stage_w["bq" + p] = np.ascontiguousarray(b[0:D].reshape(D, 1))
        stage_w["bk" + p] = np.ascontiguousarray(b[D:2 * D].reshape(D, 1))
        stage_w["bv" + p] = np.ascontiguousarray(
            b[2 * D:3 * D].reshape(H, DH, 1).astype(wdt))
    maps = []
    for i in range(N_CORES):
        m = {"xi": np.ascontiguousarray(
            x[0, :, i * RLOC:(i + 1) * RLOC, :].reshape(D, PIX))}
        m.update(stage_w)
        maps.append(m)
    return maps


def _get_runner(variant="full"):
    """Build (once) a cached jitted shard_map callable over the 8 cores."""
    rkey = "runner:" + variant
    if rkey in _CACHE:
        return _CACHE[rkey]
    import jax
    from jax.sharding import Mesh, PartitionSpec
    from jax.experimental.shard_map import shard_map
    from concourse import bass2jax as b2j

    nc = _get_nc(variant)
    b2j.install_neuronx_cc_hook()
    part_name = nc.partition_id_tensor.name if nc.partition_id_tensor else None
    in_names, out_names, out_avals, zero_outs = [], [], [], []
    for alloc in nc.m.functions[0].allocations:
        if not isinstance(alloc, mybir.MemoryLocationSet):
            continue
        name = alloc.memorylocations[0].name
        if alloc.kind == "ExternalInput":
            if name != part_name:
                in_names.append(name)
        elif alloc.kind == "ExternalOutput":
            out_names.append(name)
            shape = tuple(alloc.tensor_shape)
            dtype = mybir.dt.np(alloc.dtype)
            out_avals.append(jax.core.ShapedArray(shape, dtype))
            zero_outs.append(np.zeros(shape, dtype))
    n_params = len(in_names)
    all_names = in_names + out_names
    if part_name is not None:
        all_names = all_names + [part_name]

    def _body(*args):
        operands = list(args)
        if part_name is not None:
            operands.append(b2j.partition_id_tensor())
        outs = b2j._bass_exec_p.bind(
            *operands,
            out_avals=tuple(out_avals),
            in_names=tuple(all_names),
            out_names=tuple(out_names),
            lowering_input_output_aliases=(),
            sim_require_finite=True,
            sim_require_nnan=True,
            nc=nc,
        )
        return tuple(outs)

    devices = jax.devices()[:N_CORES]
    mesh = Mesh(np.asarray(devices), ("core",))
    specs = (PartitionSpec("core"),) * (n_params + len(out_names))
    sharded = jax.jit(
        shard_map(_body, mesh=mesh, in_specs=specs,
                  out_specs=(PartitionSpec("core"),) * len(out_names),
                  check_rep=False),
        keep_unused=True,
    )
    concat_zeros = [
        jax.device_put(
            np.zeros((N_CORES * z.shape[0], *z.shape[1:]), z.dtype),
            jax.sharding.NamedSharding(mesh, PartitionSpec("core")))
        for z in zero_outs
    ]
    _CACHE[rkey] = (sharded, in_names, out_names, out_avals, concat_zeros)
    return _CACHE[rkey]


def _run(maps):
    sharded, in_names, out_names, out_avals, concat_zeros = _get_runner()
    concat_in = [
        np.concatenate([maps[c][nm] for c in range(N_CORES)], axis=0)
        for nm in in_names
    ]
    out_arrs = sharded(*concat_in, *concat_zeros)
    return [
        {nm: np.asarray(out_arrs[i]).reshape(N_CORES, *out_avals[i].shape)[c]
         for i, nm in enumerate(out_names)}
        for c in range(N_CORES)
    ]


def kernel(x, Wr, br, Wc, bc):
    maps = _in_maps(x, Wr, br, Wc, bc)
    results = _run(maps)
    # y per core is [c, l_local, s] -> [c, s, l_local], concat over cores on l
    out = np.concatenate(
        [results[i]["y"].transpose(0, 2, 1) for i in range(N_CORES)], axis=2)
    return out[None].astype(np.float32)


# revision 33
# speedup vs baseline: 1.0582x; 1.0085x over previous
"""AxialSelfAttention2d Trainium2 kernel (8 NeuronCores).

Sharding: stage 1 (row attention, attends along L) is S-sharded (32 rows/core);
stage 2 (column attention, attends along S) is L-sharded (32 cols/core).
Between stages an AllToAll reshards out1 = x + row_out (bf16 payload).

Per-core stage structure ("rows" = s for stage 1, l for stage 2; the attended
axis is 256 long):
  - QKV 1x1-conv projection as matmuls; q/k in [chan, pix] layout (fp32r),
    v projected transposed ([pix, chan]), evacuated PSUM->SBUF on gpsimd.
  - Per (head): QK logits for both rows (fp32r, no max subtraction -- logits
    bounded ~|10|), one fused exp per row on ACT, AV accumulating into a
    [64, 512] PSUM tile (row r in columns r*256:...), ones-matmul softmax
    denominators broadcast over 64 partitions, one reciprocal + one fused
    strided DVE mul per head writing normalized outputs into staging.
  - Residual folded in on gpsimd per chunk.
DMA: all HBM-touching transfers have >=512B contiguous runs. Stage-1 staging
is [c, l, s]-ordered so a2a_in[j, c, g, l, s] group-stores are contiguous;
stage-2 x lives SBUF-resident in xbuf [c, l, s] (bf16), loaded from a2a_out
in 32 bulk contiguous DMAs; y is [c, l_local, s] (host transposes).
"""

import numpy as np
import concourse.bass as bass
import concourse.tile as tile
import concourse.mybir as mybir
from concourse import bacc
from concourse.bass_utils import run_bass_kernel_spmd

N_CORES = 8
D = 512                 # embed channels
H = 8                   # heads
DH = 64                 # head dim
S = 256
L = 256
RLOC = 32               # rows per core (s-rows stage 1, l-cols stage 2)
PIX = RLOC * 256        # 8192 pixels per core per stage
F32 = mybir.dt.float32
F32R = mybir.dt.float32r
BF16 = mybir.dt.bfloat16
ADD = mybir.AluOpType.add

_CACHE = {}


def _load_weights(nc, sb, prefix, w_ins, tag=None, bf16=False):
    """DMA weight/bias DRAM inputs into SBUF tiles. Returns dict of tiles.

    With bf16=True the w/bvr DRAM tensors are bf16 and land in the front
    half of the same (tag-aliased) f32r-sized buffers via bitcast views.
    """
    wq_d, wk_d, wv_d, bq_d, bk_d, bv_d = w_ins
    out = {}
    for wname, wd in (("wq", wq_d), ("wk", wk_d), ("wv", wv_d)):
        tiles = []
        for c4 in range(4):
            t = sb.tile([128, 512], F32R, name=f"{prefix}{wname}{c4}",
                        tag=(f"{tag}{wname}{c4}" if tag else None),
                        bufs=1)
            v = t[:].bitcast(BF16)[:, 0:512] if bf16 else t[:]
            nc.scalar.dma_start(v, wd[c4 * 128:(c4 + 1) * 128, :])
            tiles.append(v)
        out[wname] = tiles
    for bname, bd in (("bq", bq_d), ("bk", bk_d)):
        tiles = []
        for oc in range(4):
            t = sb.tile([128, 1], F32, name=f"{prefix}{bname}{oc}",
                        tag=(f"{tag}{bname}{oc}" if tag else None), bufs=1)
            nc.scalar.dma_start(t[:], bd[oc * 128:(oc + 1) * 128, :])
            tiles.append(t[:])
        out[bname] = tiles
    bvr = sb.tile([1, 512], F32R, name=f"{prefix}bvr",
                  tag=(f"{tag}bvr" if tag else None), bufs=1)
    if bf16:
        bv = bvr[:].bitcast(BF16)[:, 0:512]
        nc.scalar.dma_start(bv, bv_d.rearrange("h d one -> one (h d)"))
        out["bvr"] = bv
    else:
        nc.scalar.dma_start(bvr[:],
                            bv_d.rearrange("h d one -> one (h d)").bitcast(F32R))
        out["bvr"] = bvr[:]
    return out


def _stage(tc, nc, sb, ps, w, consts, x_get, stag_new, out_store,
           prefix):
    """One attention stage over this core's 32 rows.

    x_get(cc, chunk) -> ([128, 512] x-tile AP, is_bf16): chunk's pixels.
    stag_new(cc) -> staging tile for one 8-row group.
    out_store(cc, g, stag_tile): emit finished group staging.
    Staging layout: stage 1 [128c, 256l, 8s]; stage 2 [128c, 8l, 256s].
    The fused normalize mul writes per-head [64, 256row, 2r] views; dest
    strides differ per stage and are derived from the tile shape.
    """
    onescol, group_done = consts
    for g in range(4):                      # groups of 8 rows
        stag = [stag_new(cc) for cc in range(4)]
        for c2 in range(4):                 # 2-row chunks within group
            chunk = g * 4 + c2
            x_t = [x_get(cc, chunk) for cc in range(4)]
            # --- q/k projections: out [o-chunk 128, 512 pix] ---
            q_sb, k_sb = [], []
            for wname, bname, dst in (("wq", "bq", q_sb), ("wk", "bk", k_sb)):
                for oc in range(4):
                    pp = ps["ps"].tile([128, 512], F32, name="pp", tag="pp",
                                       bufs=2)
                    for c4 in range(4):
                        nc.tensor.matmul(
                            pp[:],
                            w[wname][c4][:, oc * 128:(oc + 1) * 128],
                            x_t[c4],
                            start=(c4 == 0), stop=(c4 == 3),
                        )
                    t = sb.tile([128, 512], F32R, name=f"{wname}o{oc}",
                                tag=f"{wname}o", bufs=4)
                    nc.scalar.activation(
                        t[:], pp[:], mybir.ActivationFunctionType.Identity,
                        bias=w[bname][oc][:],
                    )
                    dst.append(t)
            # --- v projected transposed [pix-chunk 128, 8 heads x 64],
            #     v-bias added via a K=1 ones x bvr matmul; PSUM->SBUF
            #     evacuation on gpsimd ---
            vT_sb = []
            for pc in range(4):
                pv = ps["ps"].tile([128, 512], F32, name="pp", tag="pp",
                                   bufs=2)
                for c4 in range(4):
                    nc.tensor.matmul(
                        pv[:],
                        x_t[c4][:, pc * 128:(pc + 1) * 128],
                        w["wv"][c4][:],
                        start=(c4 == 0), stop=False,
                    )
                nc.tensor.matmul(
                    pv[:], w["ones"], w["bvr"],
                    start=False, stop=True,
                )
                t = sb.tile([128, 512], F32R, name=f"vT{pc}", tag="vT",
                            bufs=4)
                with nc.allow_low_precision(reason="fp32r staging"):
                    nc.vector.tensor_copy(t[:], pv[:])
                vT_sb.append(t)

            # --- attention per head, r-paired, software-pipelined:
            #     QK+exp of head h+1 emitted before normalize of head h ---
            def emit_qkexp(h):
                m, ph = h // 2, (h % 2) * 64
                e_t = []
                for r in range(2):
                    at = ps["ps"].tile([128, 512], F32, name="at", tag="at",
                                       bufs=3)
                    for jh in range(2):
                        nc.tensor.matmul(
                            at[:, jh * 256:(jh + 1) * 256],
                            k_sb[m][ph:ph + 64,
                                    r * 256 + jh * 128:
                                    r * 256 + (jh + 1) * 128],
                            q_sb[m][ph:ph + 64, r * 256:(r + 1) * 256],
                            start=True, stop=True,
                        )
                    e = sb.tile([128, 512], F32R, name="e_t", tag="e_t",
                                bufs=3)
                    nc.scalar.activation(
                        e[:], at[:], mybir.ActivationFunctionType.Exp)
                    e_t.append(e)
                return e_t

            def emit_av(h, e_t):
                # ob rows = attn @ v (unnormalized), columns r*256:... per r;
                # dn = softmax denominators broadcast over 64 partitions
                ob = ps["ps"].tile([64, 512], F32, name="ob", tag="ob",
                                   bufs=2)
                dn = ps["ps"].tile([64, 512], F32, name="dn", tag="dn",
                                   bufs=1)
                for r in range(2):
                    for jh in range(2):
                        nc.tensor.matmul(
                            ob[:, r * 256:(r + 1) * 256],
                            vT_sb[2 * r + jh][:, h * 64:h * 64 + 64],
                            e_t[r][:, jh * 256:(jh + 1) * 256],
                            start=(jh == 0), stop=(jh == 1),
                        )
                for r in range(2):
                    for jh in range(2):
                        nc.tensor.matmul(
                            dn[:, r * 256:(r + 1) * 256],
                            onescol[:, 0:64],
                            e_t[r][:, jh * 256:(jh + 1) * 256],
                            start=(jh == 0), stop=(jh == 1),
                        )
                r_sb = sb.tile([64, 512], F32, name="r_sb", tag="r_sb",
                               bufs=2)
                nc.vector.reciprocal(r_sb[:], dn[:])
                return ob, r_sb

            def emit_norm(h, ob, r_sb):
                m, ph = h // 2, (h % 2) * 64
                # strided (row, r) views: ob/r_sb columns are r*256 + i
                st = stag[m]
                if st.shape[1] == 256:      # stage 1: [c, 256l, 8s]
                    dst = st[ph:ph + 64, :, c2 * 2:c2 * 2 + 2]
                else:                       # stage 2: [c, 8l, 256s]
                    dst = st[ph:ph + 64, c2 * 2:c2 * 2 + 2, :] \
                        .rearrange("p r i -> p i r")
                src = ob[:].rearrange("p (r i) -> p i r", r=2)
                rcv = r_sb[:].rearrange("p (r i) -> p i r", r=2)
                with nc.allow_low_precision(reason="staging dtype"):
                    nc.vector.tensor_mul(dst, src, rcv)

            pending = None
            for h in range(H):
                e_t = emit_qkexp(h)
                ob, r_sb = emit_av(h, e_t)
                if pending is not None:
                    emit_norm(*pending)
                pending = (h, ob, r_sb)
            emit_norm(*pending)
            # bulk residual: stag[cc] group slice += x (on gpsimd)
            for cc in range(4):
                st = stag[cc]
                if st.shape[1] == 256:      # stage 1
                    dst = st[:, :, c2 * 2:c2 * 2 + 2]
                    xv = x_t[cc].rearrange("c (r i) -> c i r", r=2)
                else:                       # stage 2
                    dst = st[:, c2 * 2:c2 * 2 + 2, :]
                    xv = x_t[cc].rearrange("c (r i) -> c r i", r=2)
                with nc.allow_low_precision(reason="staging dtype"):
                    nc.gpsimd.tensor_add(dst, dst, xv)
        for cc in range(4):
            out_store(cc, g, stag[cc])
        if group_done is not None:
            group_done(g)


def _build(variant="full"):
    ndev = 1 if variant == "sim1" else N_CORES
    nc = bacc.Bacc("TRN2", target_bir_lowering=False, debug=False,
                   num_devices=ndev)
    if variant == "noop":
        xi = nc.dram_tensor("xi", [128, 512], F32, kind="ExternalInput").ap()
        y = nc.dram_tensor("y", [128, 512], F32, kind="ExternalOutput").ap()
        with tile.TileContext(nc) as tc:
            with tc.tile_pool(name="sb", bufs=1) as sb:
                t = sb.tile([128, 512], F32, name="t")
                nc.sync.dma_start(t[:], xi[:])
                nc.sync.dma_start(y[:], t[:])
        nc.compile()
        return nc

    xi = nc.dram_tensor("xi", [D, PIX], F32R, kind="ExternalInput").ap()
    y = nc.dram_tensor("y", [D, RLOC, S], F32, kind="ExternalOutput").ap()
    w_ins = {}
    for p in ("1", "2"):
        ins = []
        for nm, shp in (("wq", [D, D]), ("wk", [D, D]), ("wv", [D, D]),
                        ("bq", [D, 1]), ("bk", [D, 1]), ("bv", [H, DH, 1])):
            if nm in ("bq", "bk"):
                dt = F32
            elif p == "2" and nm in ("wq", "wk", "wv", "bv"):
                dt = BF16
            else:
                dt = F32R
            ins.append(nc.dram_tensor(nm + p, shp, dt, kind="ExternalInput").ap())
        w_ins[p] = ins

    n_rep = {"full3": 3, "full8": 8, "noa2a8": 8}.get(variant, 1)
    use_a2a = variant not in ("noa2a", "noa2a8", "sim1")

    with tile.TileContext(nc) as tc:
        with tc.tile_pool(name="sb", bufs=1) as sb, \
             tc.tile_pool(name="psum", bufs=1, space="PSUM") as psp, \
             tc.tile_pool(name="dram", bufs=1, space="DRAM") as dram:
            ps = {"ps": psp, "sb": sb}
            # per-group a2a blocks: [dest core j][c][l local to j][s in group]
            a2a_in = [dram.tile([N_CORES, D, RLOC, 8], BF16, name=f"a2a_in{g}")
                      for g in range(4)]
            a2a_out = [dram.tile([N_CORES, D, RLOC, 8], BF16,
                                 name=f"a2a_out{g}") for g in range(4)]

            ones_sb = sb.tile([1, 128], F32R, name="ones_sb", bufs=1)
            nc.gpsimd.memset(ones_sb[:].bitcast(mybir.dt.uint32), 0x3F800000)
            ones_bf = sb.tile([1, 128], BF16, name="ones_bf", bufs=1)
            nc.gpsimd.memset(ones_bf[:].bitcast(mybir.dt.uint16), 0x3F80)
            onescol = sb.tile([128, 64], F32R, name="onescol", bufs=1)
            nc.gpsimd.memset(onescol[:].bitcast(mybir.dt.uint32), 0x3F800000)

            w1 = _load_weights(nc, sb, "s1", w_ins["1"], tag="w")
            w1["ones"] = ones_sb[0:1, :]
            # xbuf: stage-2 x resident [c, l local, s] in bf16
            xbuf = [sb.tile([128, RLOC, S], BF16, name=f"xbuf{cc}", bufs=1)
                    for cc in range(4)]

            # ---- stage 1: row attention, S-sharded ----
            def x_get1(cc, chunk):
                t = sb.tile([128, 512], F32R, name=f"x{cc}", tag=f"x{cc}",
                            bufs=2)
                nc.sync.dma_start(
                    t[:], xi[cc * 128:(cc + 1) * 128,
                             chunk * 512:(chunk + 1) * 512])
                return t[:]

            def stag_new1(cc):
                return ps["sb"].tile([128, 256, 8], BF16, name=f"s1g{cc}",
                                     tag=f"s1g{cc}", bufs=2)

            def out_store1(cc, g, stg):
                # [c, 256l, 8s] -> a2a_in[g][j, c, l32, s8]; contiguous
                # 512B runs per partition on the HBM side
                dst = a2a_in[g][:, cc * 128:(cc + 1) * 128, :, :] \
                    .transpose([1, 0, 2, 3])
                nc.scalar.dma_start(
                    dst, stg[:].rearrange("c (j l) s -> c j l s", j=N_CORES))

            def group_done1(g):
                # reshard group g; overlaps stage-1 compute of later groups
                if not use_a2a:
                    for j in range(N_CORES):
                        nc.gpsimd.dma_start(a2a_out[g][j], a2a_in[g][j])
                else:
                    nc.gpsimd.collective_compute(
                        "AllToAll", mybir.AluOpType.bypass,
                        replica_groups=[list(range(N_CORES))],
                        ins=[a2a_in[g].opt()], outs=[a2a_out[g].opt()],
                    )

            def load_xbuf():
                # a2a_out[g][i, c, l, s8] -> xbuf[c, l, i*32 + g*8 + s],
                # spread across three DMA queues. tile_wait_until keeps the
                # scheduler from hoisting these into the stage-1 queue
                # streams, where their wait-for-collective would block the
                # FIFO sequencers.
                engs = (nc.sync, nc.scalar, nc.gpsimd)
                n = 0
                # stagger per group: group g's loads become schedulable just
                # after collective g completes, so they drain during stage 1
                # without their collective-wait blocking a FIFO sequencer
                for g in range(4):
                    with tc.tile_wait_until(0.21 + 0.10 * g):
                        for i in range(N_CORES):
                            for cc in range(4):
                                o = i * 32 + g * 8
                                engs[n % 3].dma_start(
                                    xbuf[cc][:, :, o:o + 8],
                                    a2a_out[g][i, cc * 128:(cc + 1) * 128,
                                               :, :])
                                n += 1

            # ---- stage 2: column attention, L-sharded ----
            def x_get2(cc, chunk):
                return xbuf[cc][:, chunk * 2:chunk * 2 + 2, :] \
                    .rearrange("c r i -> c (r i)")

            def stag_new2(cc):
                return ps["sb"].tile([128, 8, 256], F32, name=f"s2g{cc}",
                                     tag=f"s2g{cc}", bufs=1)

            def out_store2(cc, g, stg):
                nc.scalar.dma_start(
                    y[cc * 128:(cc + 1) * 128, g * 8:(g + 1) * 8, :], stg[:])

            for rep in range(n_rep):
                _stage(tc, nc, sb, ps, w1, (onescol, group_done1), x_get1,
                       stag_new1, out_store1, "s1")
                load_xbuf()
                if rep == 0:
                    if n_rep == 1:
                        w2 = _load_weights(nc, sb, "s2", w_ins["2"], tag="w",
                                           bf16=True)
                    else:
                        # timing probes: stage-2 runs on bitcast views of the
                        # stage-1 weight buffers (garbage values, same timing)
                        w2 = {k: [a.bitcast(BF16)[:, 0:512] for a in v]
                              for k, v in w1.items()
                              if k in ("wq", "wk", "wv")}
                        w2["bq"], w2["bk"] = w1["bq"], w1["bk"]
                        w2["bvr"] = w1["bvr"].bitcast(BF16)[:, 0:512]
                    w2["ones"] = ones_bf[0:1, :]
                _stage(tc, nc, sb, ps, w2, (onescol, None), x_get2,
                       stag_new2, out_store2, "s2")

    nc.compile()
    return nc


def _get_nc(variant="full"):
    key = "nc:" + variant
    if key not in _CACHE:
        _CACHE[key] = _build(variant)
    return _CACHE[key]


def _in_maps(x, Wr, br, Wc, bc):
    import ml_dtypes
    x = np.asarray(x, dtype=np.float32)
    stage_w = {}
    for p, W, b in (("1", np.asarray(Wr, np.float32), np.asarray(br, np.float32)),
                    ("2", np.asarray(Wc, np.float32), np.asarray(bc, np.float32))):
        wdt = ml_dtypes.bfloat16 if p == "2" else np.float32
        stage_w["wq" + p] = np.ascontiguousarray(W[0:D].T.astype(wdt))
        stage_w["wk" + p] = np.ascontiguousarray(W[D:2 * D].T.astype(wdt))
        stage_w["wv" + p] = np.ascontiguousarray(W[2 * D:3 * D].T.astype(wdt))
        stage_w["bq" + p] = np.ascontiguousarray(b[0:D].reshape(D, 1))
        stage_w["bk" + p] = np.ascontiguousarray(b[D:2 * D].reshape(D, 1))
        stage_w["bv" + p] = np.ascontiguousarray(
            b[2 * D:3 * D].reshape(H, DH, 1).astype(wdt))
    maps = []
    for i in range(N_CORES):
        m = {"xi": np.ascontiguousarray(
            x[0, :, i * RLOC:(i + 1) * RLOC, :].reshape(D, PIX))}
        m.update(stage_w)
        maps.append(m)
    return maps


def _get_runner(variant="full"):
    """Build (once) a cached jitted shard_map callable over the 8 cores."""
    rkey = "runner:" + variant
    if rkey in _CACHE:
        return _CACHE[rkey]
    import jax
    from jax.sharding import Mesh, PartitionSpec
    from jax.experimental.shard_map import shard_map
    from concourse import bass2jax as b2j

    nc = _get_nc(variant)
    b2j.install_neuronx_cc_hook()
    part_name = nc.partition_id_tensor.name if nc.partition_id_tensor else None
    in_names, out_names, out_avals, zero_outs = [], [], [], []
    for alloc in nc.m.functions[0].allocations:
        if not isinstance(alloc, mybir.MemoryLocationSet):
            continue
        name = alloc.memorylocations[0].name
        if alloc.kind == "ExternalInput":
            if name != part_name:
                in_names.append(name)
        elif alloc.kind == "ExternalOutput":
            out_names.append(name)
            shape = tuple(alloc.tensor_shape)
            dtype = mybir.dt.np(alloc.dtype)
            out_avals.append(jax.core.ShapedArray(shape, dtype))
            zero_outs.append(np.zeros(shape, dtype))
    n_params = len(in_names)
    all_names = in_names + out_names
    if part_name is not None:
        all_names = all_names + [part_name]

    def _body(*args):
        operands = list(args)
        if part_name is not None:
            operands.append(b2j.partition_id_tensor())
        outs = b2j._bass_exec_p.bind(
            *operands,
            out_avals=tuple(out_avals),
            in_names=tuple(all_names),
            out_names=tuple(out_names),
            lowering_input_output_aliases=(),
            sim_require_finite=True,
            sim_require_nnan=True,
            nc=nc,
        )
        return tuple(outs)

    devices = jax.devices()[:N_CORES]
    mesh = Mesh(np.asarray(devices), ("core",))
    specs = (PartitionSpec("core"),) * (n_params + len(out_names))
    sharded = jax.jit(
        shard_map(_body, mesh=mesh, in_specs=specs,
                  out_specs=(PartitionSpec("core"),) * len(out_names),
                  check_rep=False),
        keep_unused=True,
    )
    concat_zeros = [
        jax.device_put(
            np.zeros((N_CORES * z.shape[0], *z.shape[1:]), z.dtype),
            jax.sharding.NamedSharding(mesh, PartitionSpec("core")))
        for z in zero_outs
    ]
    _CACHE[rkey] = (sharded, in_names, out_names, out_avals, concat_zeros)
    return _CACHE[rkey]


def _run(maps):
    sharded, in_names, out_names, out_avals, concat_zeros = _get_runner()
    concat_in = [
        np.concatenate([maps[c][nm] for c in range(N_CORES)], axis=0)
        for nm in in_names
    ]
    out_arrs = sharded(*concat_in, *concat_zeros)
    return [
        {nm: np.asarray(out_arrs[i]).reshape(N_CORES, *out_avals[i].shape)[c]
         for i, nm in enumerate(out_names)}
        for c in range(N_CORES)
    ]


def kernel(x, Wr, br, Wc, bc):
    maps = _in_maps(x, Wr, br, Wc, bc)
    results = _run(maps)
    # y per core is [c, l_local, s] -> [c, s, l_local], concat over cores on l
    out = np.concatenate(
        [results[i]["y"].transpose(0, 2, 1) for i in range(N_CORES)], axis=2)
    return out[None].astype(np.float32)
